# revision 69
# baseline (speedup 1.0000x reference)
"""LIF multicompartment refractory cell step on 8 Trainium2 NeuronCores.

Data-parallel over batch: each core handles B_LOC=512 of B=4096 rows.
On-device layout is transposed ([H, B_loc]) and fully host-preswizzled so
every DMA is a flat [128, X] transfer. The hidden/contraction dim sits on
SBUF partitions, so the GEMMs need no on-device transposes:

  vdec = v @ (G + 0.9 I).T + 0.1 i    (K=2048, f32r — leak folded into G)
  ps2  = inp @ Wi.T + z @ Wr.T        (one K=4096 accumulation chain:
                                       inp-half bf16, z-half fp8 DoubleRow)

Precision split by where the error lands:
  - coupling GEMM f32r: its error flips spikes (z_new) at the v>1
    threshold, which dominates the error budget.
  - inp@Wi bf16 except k-tiles 0..NKF-1 which run fp8-e4m3 DoubleRow
    (xf/w2f copies); z@Wr fully fp8-e4m3 DoubleRow (2 k-tiles per
    instruction, 0.5 cyc/row). This error lands on the continuous i_new
    output only (1.73e-2 on i_new, 1.39e-2 total vs the 2e-2 gate,
    measured on hardware).
  - i/rho state streams and v/i/rho outputs bf16; z output uint8.

Elementwise refractory update is mask-free via predicated copies:
  z_raw = vdec > 1;  m = rho > 0
  v_new = vdec, 0 where z_raw, v where m
  z_new = z_raw, 0 where m        (uint8; DMA'd out directly)
  rho_new = relu(rho-1), 5 where z_new   [max(rho-(rho>0),0) == relu(rho-1)]
  i_new = 0.8 i + ps2

DMA traffic (~46 MB/core) is split across the two HWDGE queues (SP and
ACT engines); a transfer occupies its issuing engine for the full
duration, so streams and weights are interleaved so the first h-tile's
operands arrive ~3.5 us in and the PE never starves (weights prefetched
3 h-tiles ahead, bufs=3 pools). The one f32->bf16 output conversion
(v_new) runs on the otherwise-idle GpSimd engine; everything else
elementwise is DVE, so the ACT engine is DMA-only and never loads an
activation table. CoreSim cost model: ~124.5 us (PE busy ~117.6 us, 94.6%).
"""
import os
import numpy as np
import ml_dtypes

import concourse.bacc as bacc
import concourse.mybir as mybir
import concourse.tile as tile
from concourse import bass_utils

B, I, H = 4096, 2048, 2048
NCORES = 8
B_LOC = B // NCORES          # 512
HT = H // 128                # 16 h-tiles
KT1 = H // 128               # 16 k-tiles per h for coupling / either gemm2 half
NKF = 2                      # inp k-tiles computed in fp8 DoubleRow
ACHUNK = 4                   # activation stream load chunks

bf16 = mybir.dt.bfloat16
fp8 = mybir.dt.float8e4
u8 = mybir.dt.uint8
nbf16 = ml_dtypes.bfloat16
nfp8 = ml_dtypes.float8_e4m3

_cache = {}


def build():
    nc = bacc.Bacc("TRN2", target_bir_lowering=False, debug=False,
                   num_devices=NCORES)
    f32 = mybir.dt.float32
    f32r = mybir.dt.float32r
    A = mybir.AluOpType
    DR = mybir.MatmulPerfMode.DoubleRow

    # activations, host layout [p, kt, b]
    vt_d = nc.dram_tensor("vt", [128, KT1 * B_LOC], f32r, kind="ExternalInput")
    xt_d = nc.dram_tensor("xt", [128, KT1 * B_LOC], bf16, kind="ExternalInput")
    zt_d = nc.dram_tensor("zt", [128, KT1, B_LOC], fp8, kind="ExternalInput")
    # fp8 copies of inp k-tiles 0-1 + matching Wi slices (all h, resident)
    xf_d = nc.dram_tensor("xf", [128, NKF, B_LOC], fp8, kind="ExternalInput")
    w2f_d = nc.dram_tensor("w2f", [128, HT * NKF, 128], fp8, kind="ExternalInput")
    # per-h state streams, same [p, ht, b] swizzle
    it_d = nc.dram_tensor("it", [128, HT * B_LOC], bf16, kind="ExternalInput")
    rt_d = nc.dram_tensor("rt", [128, HT * B_LOC], bf16, kind="ExternalInput")
    # weights pre-swizzled: [p, ht, kt, c]
    w1_d = nc.dram_tensor("w1", [128, HT * KT1 * 128], f32r, kind="ExternalInput")
    w2x_d = nc.dram_tensor("w2x", [128, HT * KT1 * 128], bf16, kind="ExternalInput")
    w2z_d = nc.dram_tensor("w2z", [128, HT * KT1, 128], fp8, kind="ExternalInput")

    zo_d = nc.dram_tensor("zo", [128, HT * B_LOC], u8, kind="ExternalOutput")
    vo_d = nc.dram_tensor("vo", [128, HT * B_LOC], bf16, kind="ExternalOutput")
    io_d = nc.dram_tensor("io", [128, HT * B_LOC], bf16, kind="ExternalOutput")
    ro_d = nc.dram_tensor("ro", [128, HT * B_LOC], bf16, kind="ExternalOutput")

    CH = KT1 * B_LOC // ACHUNK
    KCH = KT1 // ACHUNK
    LOOKAHEAD = 3

    with tile.TileContext(nc) as tc:
        with (
            tc.tile_pool(name="resid", bufs=1) as resid,
            tc.tile_pool(name="w1pool", bufs=3) as w1pool,
            tc.tile_pool(name="w2xpool", bufs=3) as w2xpool,
            tc.tile_pool(name="w2zpool", bufs=3) as w2zpool,
            tc.tile_pool(name="spool", bufs=3) as spool,
            tc.tile_pool(name="epool", bufs=3) as epool,
            tc.tile_pool(name="opool", bufs=3) as opool,
            tc.tile_pool(name="pspool", bufs=4, space="PSUM") as pspool,
        ):
            vt_sb = resid.tile([128, KT1 * B_LOC], f32r)
            xt_sb = resid.tile([128, KT1 * B_LOC], bf16)
            zt_sb = resid.tile([128, KT1, B_LOC], fp8)
            xf_sb = resid.tile([128, NKF, B_LOC], fp8)
            w2f_sb = resid.tile([128, HT * NKF, 128], fp8)
            zero_sb = resid.tile([128, B_LOC], f32)
            five_sb = resid.tile([128, B_LOC], bf16)
            zero_u8 = resid.tile([128, B_LOC], u8)
            nc.vector.memset(zero_sb[:], 0.0)
            nc.vector.memset(five_sb[:], 5.0)
            nc.vector.memset(zero_u8[:], 0)

            wtiles = {}

            def issue_w(h):
                w1_sb = w1pool.tile([128, KT1 * 128], f32r, name="w1_sb")
                w2x_sb = w2xpool.tile([128, KT1 * 128], bf16, name="w2x_sb")
                w2z_sb = w2zpool.tile([128, KT1, 128], fp8, name="w2z_sb")
                # k-tiles 0..NKF-1 come from the fp8 w2f copy instead
                nc.scalar.dma_start(
                    w2x_sb[:, NKF * 128:],
                    w2x_d[:, h * KT1 * 128 + NKF * 128:(h + 1) * KT1 * 128])
                nc.sync.dma_start(
                    w1_sb[:], w1_d[:, h * KT1 * 128:(h + 1) * KT1 * 128])
                nc.scalar.dma_start(
                    w2z_sb[:], w2z_d[:, h * KT1:(h + 1) * KT1, :])
                wtiles[h] = (w1_sb, w2x_sb, w2z_sb)

            # The PE's first chain is h0 gemm2-xt: its operands (w2x h0 on
            # ACT, xt c0 on SP) land first so the PE starts ~3.5us in (DMA
            # completion sems lag transfers by ~1.7us). SP then carries the
            # big vt stream (which gates coupling h0); ACT carries the rest
            # of h0's operands in the order the in-order PE consumes them.
            w1_h0 = w1pool.tile([128, KT1 * 128], f32r)
            w2x_h0 = w2xpool.tile([128, KT1 * 128], bf16)
            w2z_h0 = w2zpool.tile([128, KT1, 128], fp8)
            # xt streams on SP back-to-back (the PE's first ladder) while
            # ACT parallel-feeds the gemm2 weights + fp8 operands; the big
            # vt stream follows xt on SP. xt k-tiles 0..NKF-1 are never
            # read (xf replaces them).
            nc.scalar.dma_start(w2x_h0[:, NKF * 128:],
                                w2x_d[:, NKF * 128:KT1 * 128])
            XKS = [NKF, 4, 8, 12, KT1]
            for c in range(ACHUNK):
                xs = slice(XKS[c] * B_LOC, XKS[c + 1] * B_LOC)
                nc.sync.dma_start(xt_sb[:, xs], xt_d[:, xs])
            nc.scalar.dma_start(xf_sb[:], xf_d[:])
            nc.scalar.dma_start(w2f_sb[:], w2f_d[:])
            wpre = {}
            for hh in range(1, LOOKAHEAD):
                t = w2xpool.tile([128, KT1 * 128], bf16, name="w2x_sb")
                nc.scalar.dma_start(
                    t[:, NKF * 128:],
                    w2x_d[:, hh * KT1 * 128 + NKF * 128:(hh + 1) * KT1 * 128])
                wpre[hh] = t
            nc.scalar.dma_start(w1_h0[:], w1_d[:, 0:KT1 * 128])
            nc.scalar.dma_start(w2z_h0[:], w2z_d[:, 0:KT1, :])
            for c in range(ACHUNK):
                ks = slice(c * KCH, (c + 1) * KCH)
                nc.scalar.dma_start(zt_sb[:, ks, :], zt_d[:, ks, :])
                cs = slice(c * CH, (c + 1) * CH)
                nc.sync.dma_start(vt_sb[:, cs], vt_d[:, cs])
            wtiles[0] = (w1_h0, w2x_h0, w2z_h0)
            for hh in range(1, LOOKAHEAD):
                w1_sb = w1pool.tile([128, KT1 * 128], f32r, name="w1_sb")
                w2z_sb = w2zpool.tile([128, KT1, 128], fp8, name="w2z_sb")
                nc.sync.dma_start(
                    w1_sb[:], w1_d[:, hh * KT1 * 128:(hh + 1) * KT1 * 128])
                nc.scalar.dma_start(
                    w2z_sb[:], w2z_d[:, hh * KT1:(hh + 1) * KT1, :])
                wtiles[hh] = (w1_sb, wpre[hh], w2z_sb)

            for h in range(HT):
                if h + LOOKAHEAD < HT:
                    issue_w(h + LOOKAHEAD)
                w1_sb, w2x_sb, w2z_sb = wtiles.pop(h)
                pw = slice(h * B_LOC, (h + 1) * B_LOC)

                # i_new GEMM: inp-half bf16, z-half fp8 DoubleRow; coupling
                # GEMM f32r (leak folded into diagonal). gemm2 first so early
                # h-tiles run while the vt stream lands; the LAST h runs
                # coupling first so the elementwise tail (which hangs off
                # ps1) overlaps the final gemm2 instead of following it.
                ps2 = pspool.tile([128, B_LOC], f32, name="ps2")
                ps1 = pspool.tile([128, B_LOC], f32, name="ps1")

                def gemm2():
                    for k in range(NKF, KT1):
                        nc.tensor.matmul(
                            ps2[:], w2x_sb[:, k * 128:(k + 1) * 128],
                            xt_sb[:, k * B_LOC:(k + 1) * B_LOC],
                            start=(k == NKF), stop=False)
                    nc.tensor.matmul(
                        ps2[:], w2f_sb[:, h * NKF:(h + 1) * NKF, :],
                        xf_sb[:], start=False, stop=False, perf_mode=DR)
                    for j in range(KT1 // 2):
                        nc.tensor.matmul(
                            ps2[:], w2z_sb[:, 2 * j:2 * j + 2, :],
                            zt_sb[:, 2 * j:2 * j + 2, :],
                            start=False, stop=(j == KT1 // 2 - 1),
                            perf_mode=DR)

                def gemm1():
                    for k in range(KT1):
                        nc.tensor.matmul(
                            ps1[:], w1_sb[:, k * 128:(k + 1) * 128],
                            vt_sb[:, k * B_LOC:(k + 1) * B_LOC],
                            start=(k == 0), stop=(k == KT1 - 1))

                if h == HT - 1:
                    gemm1(), gemm2()
                else:
                    gemm2(), gemm1()

                v2 = vt_sb[:, pw].bitcast(f32)

                i2 = spool.tile([128, B_LOC], bf16, name="i2")
                r2 = spool.tile([128, B_LOC], bf16, name="r2")
                nc.sync.dma_start(i2[:], it_d[:, pw])
                nc.sync.dma_start(r2[:], rt_d[:, pw])

                # i_new = 0.8*i + ps2, straight to bf16
                io2 = opool.tile([128, B_LOC], bf16, name="io2")
                nc.vector.scalar_tensor_tensor(
                    io2[:], in0=i2[:], scalar=0.8, in1=ps2[:],
                    op0=A.mult, op1=A.add)
                # vdec = 0.1*i + ps1   (leak 0.9*v folded into w1 diagonal)
                vdec = epool.tile([128, B_LOC], f32, name="vdec")
                nc.vector.scalar_tensor_tensor(
                    vdec[:], in0=i2[:], scalar=0.1, in1=ps1[:],
                    op0=A.mult, op1=A.add)

                z2 = epool.tile([128, B_LOC], u8, name="z2")
                m2 = epool.tile([128, B_LOC], u8, name="m2")
                nc.vector.tensor_scalar(z2[:], vdec[:], 1.0, None, op0=A.is_gt)
                nc.vector.tensor_scalar(m2[:], r2[:], 0.0, None, op0=A.is_gt)
                # base = max(rho - 1, 0)  ==  max(rho - (rho>0), 0)
                # (bf16: rho is bf16 on input, and base IS the rho_new
                # output except where z_new spikes)
                base = epool.tile([128, B_LOC], bf16, name="base")
                nc.vector.tensor_scalar(base[:], r2[:], 1.0, 0.0,
                                        op0=A.subtract, op1=A.max)

                # v_new: spike reset, then refractory hold
                nc.vector.copy_predicated(vdec[:], z2[:], zero_sb[:])
                nc.vector.copy_predicated(vdec[:], m2[:], v2)
                # z_new: suppress spikes while refractory (uint8, final)
                nc.vector.copy_predicated(z2[:], m2[:], zero_u8[:])
                # rho_new = base, 5 where z_new
                nc.vector.copy_predicated(base[:], z2[:], five_sb[:])

                vo2 = opool.tile([128, B_LOC], bf16, name="vo2")
                nc.gpsimd.tensor_copy(vo2[:], vdec[:])

                nc.scalar.dma_start(io_d[:, pw], io2[:])
                nc.sync.dma_start(zo_d[:, pw], z2[:])
                nc.sync.dma_start(vo_d[:, pw], vo2[:])
                nc.scalar.dma_start(ro_d[:, pw], base[:])

    nc.compile()
    return nc


def _sw_act(x, dt=np.float32):
    """[B_LOC, K] -> [128, KT*B_LOC] with layout [p, kt, b]."""
    a = np.ascontiguousarray(x.T).reshape(KT1, 128, B_LOC).transpose(1, 0, 2)
    return np.ascontiguousarray(a).astype(dt).reshape(128, KT1 * B_LOC)


def _unsw(y):
    """[128, HT*B_LOC] ([p, ht, b]) -> [B_LOC, H]."""
    a = y.reshape(128, HT, B_LOC).transpose(1, 0, 2).reshape(H, B_LOC)
    return a.T


def _sw_w(WT, kt, dt=np.float32):
    """WT=[K,H] -> [128, HT*kt*128] with layout [p, ht, kt, c]."""
    a = WT.reshape(kt, 128, HT, 128)              # [k, p, h, c]
    return np.ascontiguousarray(
        a.transpose(1, 2, 0, 3)).astype(dt).reshape(128, HT * kt * 128)


def kernel(inp, z, v, i, rho, input_weights, recurrent_weights, g_coupling):
    inp = np.ascontiguousarray(inp, dtype=np.float32)
    z = np.ascontiguousarray(z, dtype=np.float32)
    v = np.ascontiguousarray(v, dtype=np.float32)
    i = np.ascontiguousarray(i, dtype=np.float32)
    rho = np.ascontiguousarray(rho, dtype=np.float32)

    if "nc" not in _cache:
        _cache["nc"] = build()
    nc = _cache["nc"]
    wkey = (id(input_weights), id(recurrent_weights), id(g_coupling))
    if _cache.get("wkey") != wkey:
        G = np.asarray(g_coupling, np.float32).T.copy()
        G[np.arange(H), np.arange(H)] += 0.9          # leak folded in
        WiT = np.ascontiguousarray(np.asarray(input_weights, np.float32).T)
        WrT = np.ascontiguousarray(np.asarray(recurrent_weights, np.float32).T)
        # Wi k-tiles 0..NKF-1 as fp8 [p, ht, kf, c] -> [128, HT*NKF, 128]
        w2f = np.ascontiguousarray(
            WiT[:NKF * 128].reshape(NKF, 128, HT, 128).transpose(1, 2, 0, 3)
        ).astype(nfp8).reshape(128, HT * NKF, 128)
        _cache["w"] = (_sw_w(G, KT1), _sw_w(WiT, KT1, nbf16),
                       _sw_w(WrT, KT1, nfp8).reshape(128, HT * KT1, 128), w2f)
        _cache["wkey"] = wkey
    w1, w2x, w2z, w2f = _cache["w"]

    in_maps = []
    for c in range(NCORES):
        s = slice(c * B_LOC, (c + 1) * B_LOC)
        in_maps.append({
            "vt": _sw_act(v[s]),
            "xt": _sw_act(inp[s], nbf16),
            "zt": _sw_act(z[s], nfp8).reshape(128, KT1, B_LOC),
            "xf": np.ascontiguousarray(
                inp[s, :NKF * 128].T.reshape(NKF, 128, B_LOC)
                .transpose(1, 0, 2)).astype(nfp8),
            "it": _sw_act(i[s], nbf16), "rt": _sw_act(rho[s], nbf16),
            "w1": w1, "w2x": w2x, "w2z": w2z, "w2f": w2f,
        })

    res = bass_utils.run_bass_kernel_spmd(
        nc, in_maps, core_ids=list(range(NCORES)),
        trace=bool(int(os.environ.get("LIF_TRACE", "0"))),
    )
    _cache["last_results"] = res

    outs = []
    for name in ["zo", "vo", "io", "ro"]:
        full = np.empty((B, H), np.float32)
        for c in range(NCORES):
            full[c * B_LOC:(c + 1) * B_LOC] = _unsw(
                res.results[c][name].astype(np.float32))
        outs.append(full)
    return np.stack(outs)


# revision 71
# speedup vs baseline: 1.0242x; 1.0242x over previous
"""LIF multicompartment refractory cell step on 8 Trainium2 NeuronCores.

Data-parallel over batch: each core handles B_LOC=512 of B=4096 rows.
On-device layout is transposed ([H, B_loc]) and fully host-preswizzled so
every DMA is a flat [128, X] transfer. The hidden/contraction dim sits on
SBUF partitions, so the GEMMs need no on-device transposes:

  vdec = v @ (G + 0.9 I).T + 0.1 i    (K=2048, f32r — leak folded into G)
  ps2  = inp @ Wi.T + z @ Wr.T        (one K=4096 accumulation chain:
                                       inp-half bf16, z-half fp8 DoubleRow)

Precision split by where the error lands:
  - coupling GEMM f32r: its error flips spikes (z_new) at the v>1
    threshold, which dominates the error budget.
  - inp@Wi bf16 except k-tiles 0..NKF-1 which run fp8-e4m3 DoubleRow
    (xf/w2f copies); z@Wr fully fp8-e4m3 DoubleRow (2 k-tiles per
    instruction, 0.5 cyc/row). This error lands on the continuous i_new
    output only (1.73e-2 on i_new, 1.39e-2 total vs the 2e-2 gate,
    measured on hardware).
  - i/rho state streams and v/i/rho outputs bf16; z output uint8.

Elementwise refractory update is mask-free via predicated copies:
  z_raw = vdec > 1;  m = rho > 0
  v_new = vdec, 0 where z_raw, v where m
  z_new = z_raw, 0 where m        (uint8; DMA'd out directly)
  rho_new = relu(rho-1), 5 where z_new   [max(rho-(rho>0),0) == relu(rho-1)]
  i_new = 0.8 i + ps2

DMA traffic (~46 MB/core) is split across the two HWDGE queues (SP and
ACT engines); a transfer occupies its issuing engine for the full
duration, so streams and weights are interleaved so the first h-tile's
operands arrive ~3.5 us in and the PE never starves (weights prefetched
3 h-tiles ahead, bufs=3 pools). The one f32->bf16 output conversion
(v_new) runs on the otherwise-idle GpSimd engine; everything else
elementwise is DVE, so the ACT engine is DMA-only and never loads an
activation table. CoreSim cost model: ~124.5 us (PE busy ~117.6 us, 94.6%).
"""
import os
import numpy as np
import ml_dtypes

import concourse.bacc as bacc
import concourse.mybir as mybir
import concourse.tile as tile
from concourse import bass_utils

B, I, H = 4096, 2048, 2048
NCORES = 8
B_LOC = B // NCORES          # 512
HT = H // 128                # 16 h-tiles
KT1 = H // 128               # 16 k-tiles per h for coupling / either gemm2 half
NKF = 4                      # inp k-tiles computed in fp8 DoubleRow
ACHUNK = 4                   # activation stream load chunks

bf16 = mybir.dt.bfloat16
fp8 = mybir.dt.float8e4
u8 = mybir.dt.uint8
nbf16 = ml_dtypes.bfloat16
nfp8 = ml_dtypes.float8_e4m3

_cache = {}


def build():
    nc = bacc.Bacc("TRN2", target_bir_lowering=False, debug=False,
                   num_devices=NCORES)
    f32 = mybir.dt.float32
    f32r = mybir.dt.float32r
    A = mybir.AluOpType
    DR = mybir.MatmulPerfMode.DoubleRow

    # activations, host layout [p, kt, b]
    vt_d = nc.dram_tensor("vt", [128, KT1 * B_LOC], f32r, kind="ExternalInput")
    xt_d = nc.dram_tensor("xt", [128, KT1 * B_LOC], bf16, kind="ExternalInput")
    zt_d = nc.dram_tensor("zt", [128, KT1, B_LOC], fp8, kind="ExternalInput")
    # fp8 copies of inp k-tiles 0-1 + matching Wi slices (all h, resident)
    xf_d = nc.dram_tensor("xf", [128, NKF, B_LOC], fp8, kind="ExternalInput")
    w2f_d = nc.dram_tensor("w2f", [128, HT * NKF, 128], fp8, kind="ExternalInput")
    # per-h state streams, same [p, ht, b] swizzle
    it_d = nc.dram_tensor("it", [128, HT * B_LOC], bf16, kind="ExternalInput")
    i2t_d = nc.dram_tensor("i2t", [128, HT * B_LOC], bf16, kind="ExternalInput")
    rt_d = nc.dram_tensor("rt", [128, HT * B_LOC], bf16, kind="ExternalInput")
    # weights pre-swizzled: [p, ht, kt, c]
    w1_d = nc.dram_tensor("w1", [128, HT * KT1 * 128], f32r, kind="ExternalInput")
    w2x_d = nc.dram_tensor("w2x", [128, HT * KT1 * 128], bf16, kind="ExternalInput")
    w2z_d = nc.dram_tensor("w2z", [128, HT * KT1, 128], fp8, kind="ExternalInput")

    zo_d = nc.dram_tensor("zo", [128, HT * B_LOC], u8, kind="ExternalOutput")
    vo_d = nc.dram_tensor("vo", [128, HT * B_LOC], bf16, kind="ExternalOutput")
    io_d = nc.dram_tensor("io", [128, HT * B_LOC], bf16, kind="ExternalOutput")
    ro_d = nc.dram_tensor("ro", [128, HT * B_LOC], bf16, kind="ExternalOutput")

    CH = KT1 * B_LOC // ACHUNK
    KCH = KT1 // ACHUNK
    LOOKAHEAD = 3

    with tile.TileContext(nc) as tc:
        with (
            tc.tile_pool(name="resid", bufs=1) as resid,
            tc.tile_pool(name="w1pool", bufs=3) as w1pool,
            tc.tile_pool(name="w2xpool", bufs=3) as w2xpool,
            tc.tile_pool(name="w2zpool", bufs=3) as w2zpool,
            tc.tile_pool(name="spool", bufs=3) as spool,
            tc.tile_pool(name="epool", bufs=3) as epool,
            tc.tile_pool(name="opool", bufs=3) as opool,
            tc.tile_pool(name="pspool", bufs=4, space="PSUM") as pspool,
        ):
            vt_sb = resid.tile([128, KT1 * B_LOC], f32r)
            xt_sb = resid.tile([128, KT1 * B_LOC], bf16)
            zt_sb = resid.tile([128, KT1, B_LOC], fp8)
            xf_sb = resid.tile([128, NKF, B_LOC], fp8)
            w2f_sb = resid.tile([128, HT * NKF, 128], fp8)
            zero_sb = resid.tile([128, B_LOC], f32)
            five_sb = resid.tile([128, B_LOC], bf16)
            zero_u8 = resid.tile([128, B_LOC], u8)
            nc.vector.memset(zero_sb[:], 0.0)
            nc.vector.memset(five_sb[:], 5.0)
            nc.vector.memset(zero_u8[:], 0)

            wtiles = {}

            def issue_w(h):
                w1_sb = w1pool.tile([128, KT1 * 128], f32r, name="w1_sb")
                w2x_sb = w2xpool.tile([128, KT1 * 128], bf16, name="w2x_sb")
                w2z_sb = w2zpool.tile([128, KT1, 128], fp8, name="w2z_sb")
                # k-tiles 0..NKF-1 come from the fp8 w2f copy instead
                nc.scalar.dma_start(
                    w2x_sb[:, NKF * 128:],
                    w2x_d[:, h * KT1 * 128 + NKF * 128:(h + 1) * KT1 * 128])
                nc.sync.dma_start(
                    w1_sb[:], w1_d[:, h * KT1 * 128:(h + 1) * KT1 * 128])
                nc.scalar.dma_start(
                    w2z_sb[:], w2z_d[:, h * KT1:(h + 1) * KT1, :])
                wtiles[h] = (w1_sb, w2x_sb, w2z_sb)

            # The PE's first chain is h0 gemm2-xt: its operands (w2x h0 on
            # ACT, xt c0 on SP) land first so the PE starts ~3.5us in (DMA
            # completion sems lag transfers by ~1.7us). SP then carries the
            # big vt stream (which gates coupling h0); ACT carries the rest
            # of h0's operands in the order the in-order PE consumes them.
            w1_h0 = w1pool.tile([128, KT1 * 128], f32r)
            w2x_h0 = w2xpool.tile([128, KT1 * 128], bf16)
            w2z_h0 = w2zpool.tile([128, KT1, 128], fp8)
            # xt streams on SP back-to-back (the PE's first ladder) while
            # ACT parallel-feeds the gemm2 weights + fp8 operands; the big
            # vt stream follows xt on SP. xt k-tiles 0..NKF-1 are never
            # read (xf replaces them).
            nc.scalar.dma_start(w2x_h0[:, NKF * 128:],
                                w2x_d[:, NKF * 128:KT1 * 128])
            XKS = [NKF, 6, 10, 14, KT1]
            for c in range(ACHUNK):
                xs = slice(XKS[c] * B_LOC, XKS[c + 1] * B_LOC)
                nc.sync.dma_start(xt_sb[:, xs], xt_d[:, xs])
            nc.scalar.dma_start(xf_sb[:], xf_d[:])
            nc.scalar.dma_start(w2f_sb[:], w2f_d[:])
            wpre = {}
            for hh in range(1, LOOKAHEAD):
                t = w2xpool.tile([128, KT1 * 128], bf16, name="w2x_sb")
                nc.scalar.dma_start(
                    t[:, NKF * 128:],
                    w2x_d[:, hh * KT1 * 128 + NKF * 128:(hh + 1) * KT1 * 128])
                wpre[hh] = t
            nc.scalar.dma_start(w1_h0[:], w1_d[:, 0:KT1 * 128])
            nc.scalar.dma_start(w2z_h0[:], w2z_d[:, 0:KT1, :])
            for c in range(ACHUNK):
                ks = slice(c * KCH, (c + 1) * KCH)
                nc.scalar.dma_start(zt_sb[:, ks, :], zt_d[:, ks, :])
                cs = slice(c * CH, (c + 1) * CH)
                nc.sync.dma_start(vt_sb[:, cs], vt_d[:, cs])
            wtiles[0] = (w1_h0, w2x_h0, w2z_h0)
            for hh in range(1, LOOKAHEAD):
                w1_sb = w1pool.tile([128, KT1 * 128], f32r, name="w1_sb")
                w2z_sb = w2zpool.tile([128, KT1, 128], fp8, name="w2z_sb")
                nc.sync.dma_start(
                    w1_sb[:], w1_d[:, hh * KT1 * 128:(hh + 1) * KT1 * 128])
                nc.scalar.dma_start(
                    w2z_sb[:], w2z_d[:, hh * KT1:(hh + 1) * KT1, :])
                wtiles[hh] = (w1_sb, wpre[hh], w2z_sb)

            for h in range(HT):
                if h + LOOKAHEAD < HT:
                    issue_w(h + LOOKAHEAD)
                w1_sb, w2x_sb, w2z_sb = wtiles.pop(h)
                pw = slice(h * B_LOC, (h + 1) * B_LOC)

                # i_new GEMM: inp-half bf16, z-half fp8 DoubleRow; coupling
                # GEMM f32r (leak folded into diagonal). gemm2 first so early
                # h-tiles run while the vt stream lands; the LAST h runs
                # coupling first so the elementwise tail (which hangs off
                # ps1) overlaps the final gemm2 instead of following it.
                ps2 = pspool.tile([128, B_LOC], f32, name="ps2")
                ps1 = pspool.tile([128, B_LOC], f32, name="ps1")

                def gemm2():
                    for k in range(NKF, KT1):
                        nc.tensor.matmul(
                            ps2[:], w2x_sb[:, k * 128:(k + 1) * 128],
                            xt_sb[:, k * B_LOC:(k + 1) * B_LOC],
                            start=(k == NKF), stop=False)
                    for jf in range(NKF // 2):
                        nc.tensor.matmul(
                            ps2[:],
                            w2f_sb[:, h * NKF + 2 * jf:h * NKF + 2 * jf + 2, :],
                            xf_sb[:, 2 * jf:2 * jf + 2, :],
                            start=False, stop=False, perf_mode=DR)
                    for j in range(KT1 // 2):
                        nc.tensor.matmul(
                            ps2[:], w2z_sb[:, 2 * j:2 * j + 2, :],
                            zt_sb[:, 2 * j:2 * j + 2, :],
                            start=False, stop=(j == KT1 // 2 - 1),
                            perf_mode=DR)

                def gemm1():
                    for k in range(KT1):
                        nc.tensor.matmul(
                            ps1[:], w1_sb[:, k * 128:(k + 1) * 128],
                            vt_sb[:, k * B_LOC:(k + 1) * B_LOC],
                            start=(k == 0), stop=(k == KT1 - 1))

                if h == HT - 1:
                    gemm1(), gemm2()
                else:
                    gemm2(), gemm1()

                v2 = vt_sb[:, pw].bitcast(f32)

                i2 = spool.tile([128, B_LOC], bf16, name="i2")
                i3 = spool.tile([128, B_LOC], bf16, name="i3")
                r2 = spool.tile([128, B_LOC], bf16, name="r2")
                nc.sync.dma_start(i2[:], it_d[:, pw])
                nc.sync.dma_start(i3[:], i2t_d[:, pw])
                nc.sync.dma_start(r2[:], rt_d[:, pw])

                # i_new = 0.8*i + ps2, straight to bf16
                io2 = opool.tile([128, B_LOC], bf16, name="io2")
                nc.vector.scalar_tensor_tensor(
                    io2[:], in0=i3[:], scalar=0.8, in1=ps2[:],
                    op0=A.mult, op1=A.add)
                # vdec = 0.1*i + ps1   (leak 0.9*v folded into w1 diagonal)
                vdec = epool.tile([128, B_LOC], f32, name="vdec")
                nc.vector.scalar_tensor_tensor(
                    vdec[:], in0=i2[:], scalar=0.1, in1=ps1[:],
                    op0=A.mult, op1=A.add)

                z2 = epool.tile([128, B_LOC], u8, name="z2")
                m2 = epool.tile([128, B_LOC], u8, name="m2")
                nc.vector.tensor_scalar(z2[:], vdec[:], 1.0, None, op0=A.is_gt)
                nc.vector.tensor_scalar(m2[:], r2[:], 0.0, None, op0=A.is_gt)
                # base = max(rho - 1, 0)  ==  max(rho - (rho>0), 0)
                # (bf16: rho is bf16 on input, and base IS the rho_new
                # output except where z_new spikes)
                base = epool.tile([128, B_LOC], bf16, name="base")
                nc.vector.tensor_scalar(base[:], r2[:], 1.0, 0.0,
                                        op0=A.subtract, op1=A.max)

                # v_new: spike reset, then refractory hold
                nc.vector.copy_predicated(vdec[:], z2[:], zero_sb[:])
                nc.vector.copy_predicated(vdec[:], m2[:], v2)
                # z_new: suppress spikes while refractory (uint8, final)
                nc.vector.copy_predicated(z2[:], m2[:], zero_u8[:])
                # rho_new = base, 5 where z_new
                nc.vector.copy_predicated(base[:], z2[:], five_sb[:])

                vo2 = opool.tile([128, B_LOC], bf16, name="vo2")
                nc.gpsimd.tensor_copy(vo2[:], vdec[:])

                nc.scalar.dma_start(io_d[:, pw], io2[:])
                nc.sync.dma_start(zo_d[:, pw], z2[:])
                nc.sync.dma_start(vo_d[:, pw], vo2[:])
                nc.scalar.dma_start(ro_d[:, pw], base[:])

    nc.compile()
    return nc


def _sw_act(x, dt=np.float32):
    """[B_LOC, K] -> [128, KT*B_LOC] with layout [p, kt, b]."""
    a = np.ascontiguousarray(x.T).reshape(KT1, 128, B_LOC).transpose(1, 0, 2)
    return np.ascontiguousarray(a).astype(dt).reshape(128, KT1 * B_LOC)


def _unsw(y):
    """[128, HT*B_LOC] ([p, ht, b]) -> [B_LOC, H]."""
    a = y.reshape(128, HT, B_LOC).transpose(1, 0, 2).reshape(H, B_LOC)
    return a.T


def _sw_w(WT, kt, dt=np.float32):
    """WT=[K,H] -> [128, HT*kt*128] with layout [p, ht, kt, c]."""
    a = WT.reshape(kt, 128, HT, 128)              # [k, p, h, c]
    return np.ascontiguousarray(
        a.transpose(1, 2, 0, 3)).astype(dt).reshape(128, HT * kt * 128)


def kernel(inp, z, v, i, rho, input_weights, recurrent_weights, g_coupling):
    inp = np.ascontiguousarray(inp, dtype=np.float32)
    z = np.ascontiguousarray(z, dtype=np.float32)
    v = np.ascontiguousarray(v, dtype=np.float32)
    i = np.ascontiguousarray(i, dtype=np.float32)
    rho = np.ascontiguousarray(rho, dtype=np.float32)

    if "nc" not in _cache:
        _cache["nc"] = build()
    nc = _cache["nc"]
    wkey = (id(input_weights), id(recurrent_weights), id(g_coupling))
    if _cache.get("wkey") != wkey:
        G = np.asarray(g_coupling, np.float32).T.copy()
        G[np.arange(H), np.arange(H)] += 0.9          # leak folded in
        WiT = np.ascontiguousarray(np.asarray(input_weights, np.float32).T)
        WrT = np.ascontiguousarray(np.asarray(recurrent_weights, np.float32).T)
        # Wi k-tiles 0..NKF-1 as fp8 [p, ht, kf, c] -> [128, HT*NKF, 128]
        w2f = np.ascontiguousarray(
            WiT[:NKF * 128].reshape(NKF, 128, HT, 128).transpose(1, 2, 0, 3)
        ).astype(nfp8).reshape(128, HT * NKF, 128)
        # z ships centered (z-0.5, halves fp8 quantization error); the
        # bias 0.5*colsum(Wr_fp8) folds into the i_new-path i stream
        zbias = 0.5 * WrT.astype(nfp8).astype(np.float32).sum(axis=0)
        _cache["w"] = (_sw_w(G, KT1), _sw_w(WiT, KT1, nbf16),
                       _sw_w(WrT, KT1, nfp8).reshape(128, HT * KT1, 128),
                       w2f, zbias)
        _cache["wkey"] = wkey
    w1, w2x, w2z, w2f, zbias = _cache["w"]

    in_maps = []
    for c in range(NCORES):
        s = slice(c * B_LOC, (c + 1) * B_LOC)
        in_maps.append({
            "vt": _sw_act(v[s]),
            "xt": _sw_act(inp[s], nbf16),
            "zt": _sw_act(z[s] - 0.5, nfp8).reshape(128, KT1, B_LOC),
            "xf": np.ascontiguousarray(
                inp[s, :NKF * 128].T.reshape(NKF, 128, B_LOC)
                .transpose(1, 0, 2)).astype(nfp8),
            "it": _sw_act(i[s], nbf16),
            "i2t": _sw_act(i[s] + zbias[None, :] / 0.8, nbf16),
            "rt": _sw_act(rho[s], nbf16),
            "w1": w1, "w2x": w2x, "w2z": w2z, "w2f": w2f,
        })

    res = bass_utils.run_bass_kernel_spmd(
        nc, in_maps, core_ids=list(range(NCORES)),
        trace=bool(int(os.environ.get("LIF_TRACE", "0"))),
    )
    _cache["last_results"] = res

    outs = []
    for name in ["zo", "vo", "io", "ro"]:
        full = np.empty((B, H), np.float32)
        for c in range(NCORES):
            full[c * B_LOC:(c + 1) * B_LOC] = _unsw(
                res.results[c][name].astype(np.float32))
        outs.append(full)
    return np.stack(outs)


# revision 73
# speedup vs baseline: 1.0259x; 1.0017x over previous
"""LIF multicompartment refractory cell step on 8 Trainium2 NeuronCores.

Data-parallel over batch: each core handles B_LOC=512 of B=4096 rows.
On-device layout is transposed ([H, B_loc]) and fully host-preswizzled so
every DMA is a flat [128, X] transfer. The hidden/contraction dim sits on
SBUF partitions, so the GEMMs need no on-device transposes:

  vdec = v @ (G + 0.9 I).T + 0.1 i    (K=2048, f32r — leak folded into G)
  ps2  = inp @ Wi.T + z @ Wr.T        (one K=4096 accumulation chain:
                                       inp-half bf16, z-half fp8 DoubleRow)

Precision split by where the error lands:
  - coupling GEMM f32r: its error flips spikes (z_new) at the v>1
    threshold, which dominates the error budget.
  - inp@Wi bf16 except k-tiles 0..NKF-1 which run fp8-e4m3 DoubleRow
    (xf/w2f copies); z@Wr fully fp8-e4m3 DoubleRow (2 k-tiles per
    instruction, 0.5 cyc/row). z ships CENTERED (z-0.5), halving its fp8
    quantization error; the bias 0.5*colsum(Wr_fp8) folds into a second
    host-prepared i stream (i2t) used only by the i_new path. This error
    lands on the continuous i_new output only (1.81e-2 on i_new, 1.43e-2
    total vs the 2e-2 gate, measured on hardware).
  - i/rho state streams and v/i/rho outputs bf16; z output uint8.

Elementwise refractory update is mask-free via predicated copies:
  z_raw = vdec > 1;  m = rho > 0
  v_new = vdec, 0 where z_raw, v where m
  z_new = z_raw, 0 where m        (uint8; DMA'd out directly)
  rho_new = relu(rho-1), 5 where z_new   [max(rho-(rho>0),0) == relu(rho-1)]
  i_new = 0.8 i + ps2

DMA traffic (~46 MB/core) is split across the two HWDGE queues (SP and
ACT engines); a transfer occupies its issuing engine for the full
duration, so streams and weights are interleaved so the first h-tile's
operands arrive ~3.5 us in and the PE never starves (weights prefetched
3 h-tiles ahead, bufs=3 pools). The one f32->bf16 output conversion
(v_new) runs on the otherwise-idle GpSimd engine; everything else
elementwise is DVE, so the ACT engine is DMA-only and never loads an
activation table. CoreSim cost model: ~121.6 us (PE busy ~112.5 us).
"""
import os
import numpy as np
import ml_dtypes

import concourse.bacc as bacc
import concourse.mybir as mybir
import concourse.tile as tile
from concourse import bass_utils

B, I, H = 4096, 2048, 2048
NCORES = 8
B_LOC = B // NCORES          # 512
HT = H // 128                # 16 h-tiles
KT1 = H // 128               # 16 k-tiles per h for coupling / either gemm2 half
NKF = 4                      # inp k-tiles computed in fp8 DoubleRow
ACHUNK = 4                   # activation stream load chunks

bf16 = mybir.dt.bfloat16
fp8 = mybir.dt.float8e4
u8 = mybir.dt.uint8
nbf16 = ml_dtypes.bfloat16
nfp8 = ml_dtypes.float8_e4m3

_cache = {}


def build():
    nc = bacc.Bacc("TRN2", target_bir_lowering=False, debug=False,
                   num_devices=NCORES)
    f32 = mybir.dt.float32
    f32r = mybir.dt.float32r
    A = mybir.AluOpType
    DR = mybir.MatmulPerfMode.DoubleRow

    # activations, host layout [p, kt, b]
    vt_d = nc.dram_tensor("vt", [128, KT1 * B_LOC], f32r, kind="ExternalInput")
    xt_d = nc.dram_tensor("xt", [128, KT1 * B_LOC], bf16, kind="ExternalInput")
    zt_d = nc.dram_tensor("zt", [128, KT1, B_LOC], fp8, kind="ExternalInput")
    # fp8 copies of inp k-tiles 0-1 + matching Wi slices (all h, resident)
    xf_d = nc.dram_tensor("xf", [128, NKF, B_LOC], fp8, kind="ExternalInput")
    w2f_d = nc.dram_tensor("w2f", [128, HT * NKF, 128], fp8, kind="ExternalInput")
    # per-h state streams, same [p, ht, b] swizzle
    it_d = nc.dram_tensor("it", [128, HT * B_LOC], bf16, kind="ExternalInput")
    i2t_d = nc.dram_tensor("i2t", [128, HT * B_LOC], bf16, kind="ExternalInput")
    rt_d = nc.dram_tensor("rt", [128, HT * B_LOC], bf16, kind="ExternalInput")
    # weights pre-swizzled: [p, ht, kt, c]
    w1_d = nc.dram_tensor("w1", [128, HT * KT1 * 128], f32r, kind="ExternalInput")
    w2x_d = nc.dram_tensor("w2x", [128, HT * KT1 * 128], bf16, kind="ExternalInput")
    w2z_d = nc.dram_tensor("w2z", [128, HT * KT1, 128], fp8, kind="ExternalInput")

    zo_d = nc.dram_tensor("zo", [128, HT * B_LOC], u8, kind="ExternalOutput")
    vo_d = nc.dram_tensor("vo", [128, HT * B_LOC], bf16, kind="ExternalOutput")
    io_d = nc.dram_tensor("io", [128, HT * B_LOC], bf16, kind="ExternalOutput")
    ro_d = nc.dram_tensor("ro", [128, HT * B_LOC], bf16, kind="ExternalOutput")

    CH = KT1 * B_LOC // ACHUNK
    KCH = KT1 // ACHUNK
    LOOKAHEAD = 3

    with tile.TileContext(nc) as tc:
        with (
            tc.tile_pool(name="resid", bufs=1) as resid,
            tc.tile_pool(name="w1pool", bufs=3) as w1pool,
            tc.tile_pool(name="w2xpool", bufs=3) as w2xpool,
            tc.tile_pool(name="w2zpool", bufs=3) as w2zpool,
            tc.tile_pool(name="spool", bufs=3) as spool,
            tc.tile_pool(name="epool", bufs=3) as epool,
            tc.tile_pool(name="opool", bufs=3) as opool,
            tc.tile_pool(name="pspool", bufs=4, space="PSUM") as pspool,
        ):
            vt_sb = resid.tile([128, KT1 * B_LOC], f32r)
            xt_sb = resid.tile([128, KT1 * B_LOC], bf16)
            zt_sb = resid.tile([128, KT1, B_LOC], fp8)
            xf_sb = resid.tile([128, NKF, B_LOC], fp8)
            w2f_sb = resid.tile([128, HT * NKF, 128], fp8)
            zero_sb = resid.tile([128, B_LOC], f32)
            five_sb = resid.tile([128, B_LOC], bf16)
            zero_u8 = resid.tile([128, B_LOC], u8)
            nc.vector.memset(zero_sb[:], 0.0)
            nc.vector.memset(five_sb[:], 5.0)
            nc.vector.memset(zero_u8[:], 0)

            wtiles = {}

            def issue_w(h):
                w1_sb = w1pool.tile([128, KT1 * 128], f32r, name="w1_sb")
                w2x_sb = w2xpool.tile([128, KT1 * 128], bf16, name="w2x_sb")
                w2z_sb = w2zpool.tile([128, KT1, 128], fp8, name="w2z_sb")
                # k-tiles 0..NKF-1 come from the fp8 w2f copy instead
                nc.scalar.dma_start(
                    w2x_sb[:, NKF * 128:],
                    w2x_d[:, h * KT1 * 128 + NKF * 128:(h + 1) * KT1 * 128])
                nc.sync.dma_start(
                    w1_sb[:], w1_d[:, h * KT1 * 128:(h + 1) * KT1 * 128])
                nc.scalar.dma_start(
                    w2z_sb[:], w2z_d[:, h * KT1:(h + 1) * KT1, :])
                wtiles[h] = (w1_sb, w2x_sb, w2z_sb)

            # The PE's first chain is h0 gemm2-xt: its operands (w2x h0 on
            # ACT, xt c0 on SP) land first so the PE starts ~3.5us in (DMA
            # completion sems lag transfers by ~1.7us). SP then carries the
            # big vt stream (which gates coupling h0); ACT carries the rest
            # of h0's operands in the order the in-order PE consumes them.
            w1_h0 = w1pool.tile([128, KT1 * 128], f32r)
            w2x_h0 = w2xpool.tile([128, KT1 * 128], bf16)
            w2z_h0 = w2zpool.tile([128, KT1, 128], fp8)
            # xt streams on SP back-to-back (the PE's first ladder) while
            # ACT parallel-feeds the gemm2 weights + fp8 operands; the big
            # vt stream follows xt on SP. xt k-tiles 0..NKF-1 are never
            # read (xf replaces them).
            nc.scalar.dma_start(w2x_h0[:, NKF * 128:],
                                w2x_d[:, NKF * 128:KT1 * 128])
            XKS = [NKF, 6, 10, 14, KT1]
            for c in range(ACHUNK):
                xs = slice(XKS[c] * B_LOC, XKS[c + 1] * B_LOC)
                nc.sync.dma_start(xt_sb[:, xs], xt_d[:, xs])
            wpre = {}
            for hh in range(1, LOOKAHEAD):
                t = w2xpool.tile([128, KT1 * 128], bf16, name="w2x_sb")
                nc.scalar.dma_start(
                    t[:, NKF * 128:],
                    w2x_d[:, hh * KT1 * 128 + NKF * 128:(hh + 1) * KT1 * 128])
                wpre[hh] = t
            nc.scalar.dma_start(xf_sb[:], xf_d[:])
            nc.scalar.dma_start(w2f_sb[:, 0:LOOKAHEAD * NKF, :],
                                w2f_d[:, 0:LOOKAHEAD * NKF, :])
            nc.scalar.dma_start(w1_h0[:], w1_d[:, 0:KT1 * 128])
            nc.scalar.dma_start(w2z_h0[:], w2z_d[:, 0:KT1, :])
            for c in range(ACHUNK):
                ks = slice(c * KCH, (c + 1) * KCH)
                nc.scalar.dma_start(zt_sb[:, ks, :], zt_d[:, ks, :])
                cs = slice(c * CH, (c + 1) * CH)
                nc.sync.dma_start(vt_sb[:, cs], vt_d[:, cs])
            nc.scalar.dma_start(w2f_sb[:, LOOKAHEAD * NKF:, :],
                                w2f_d[:, LOOKAHEAD * NKF:, :])
            wtiles[0] = (w1_h0, w2x_h0, w2z_h0)
            for hh in range(1, LOOKAHEAD):
                w1_sb = w1pool.tile([128, KT1 * 128], f32r, name="w1_sb")
                w2z_sb = w2zpool.tile([128, KT1, 128], fp8, name="w2z_sb")
                nc.sync.dma_start(
                    w1_sb[:], w1_d[:, hh * KT1 * 128:(hh + 1) * KT1 * 128])
                nc.scalar.dma_start(
                    w2z_sb[:], w2z_d[:, hh * KT1:(hh + 1) * KT1, :])
                wtiles[hh] = (w1_sb, wpre[hh], w2z_sb)

            for h in range(HT):
                if h + LOOKAHEAD < HT:
                    issue_w(h + LOOKAHEAD)
                w1_sb, w2x_sb, w2z_sb = wtiles.pop(h)
                pw = slice(h * B_LOC, (h + 1) * B_LOC)

                # i_new GEMM: inp-half bf16, z-half fp8 DoubleRow; coupling
                # GEMM f32r (leak folded into diagonal). gemm2 first so early
                # h-tiles run while the vt stream lands; the LAST h runs
                # coupling first so the elementwise tail (which hangs off
                # ps1) overlaps the final gemm2 instead of following it.
                ps2 = pspool.tile([128, B_LOC], f32, name="ps2")
                ps1 = pspool.tile([128, B_LOC], f32, name="ps1")

                def gemm2():
                    for k in range(NKF, KT1):
                        nc.tensor.matmul(
                            ps2[:], w2x_sb[:, k * 128:(k + 1) * 128],
                            xt_sb[:, k * B_LOC:(k + 1) * B_LOC],
                            start=(k == NKF), stop=False)
                    for jf in range(NKF // 2):
                        nc.tensor.matmul(
                            ps2[:],
                            w2f_sb[:, h * NKF + 2 * jf:h * NKF + 2 * jf + 2, :],
                            xf_sb[:, 2 * jf:2 * jf + 2, :],
                            start=False, stop=False, perf_mode=DR)
                    for j in range(KT1 // 2):
                        nc.tensor.matmul(
                            ps2[:], w2z_sb[:, 2 * j:2 * j + 2, :],
                            zt_sb[:, 2 * j:2 * j + 2, :],
                            start=False, stop=(j == KT1 // 2 - 1),
                            perf_mode=DR)

                def gemm1():
                    for k in range(KT1):
                        nc.tensor.matmul(
                            ps1[:], w1_sb[:, k * 128:(k + 1) * 128],
                            vt_sb[:, k * B_LOC:(k + 1) * B_LOC],
                            start=(k == 0), stop=(k == KT1 - 1))

                if h == HT - 1:
                    gemm1(), gemm2()
                else:
                    gemm2(), gemm1()

                v2 = vt_sb[:, pw].bitcast(f32)

                i2 = spool.tile([128, B_LOC], bf16, name="i2")
                i3 = spool.tile([128, B_LOC], bf16, name="i3")
                r2 = spool.tile([128, B_LOC], bf16, name="r2")
                nc.sync.dma_start(i2[:], it_d[:, pw])
                nc.sync.dma_start(i3[:], i2t_d[:, pw])
                nc.sync.dma_start(r2[:], rt_d[:, pw])

                # i_new = 0.8*i + ps2, straight to bf16
                io2 = opool.tile([128, B_LOC], bf16, name="io2")
                nc.vector.scalar_tensor_tensor(
                    io2[:], in0=i3[:], scalar=0.8, in1=ps2[:],
                    op0=A.mult, op1=A.add)
                # vdec = 0.1*i + ps1   (leak 0.9*v folded into w1 diagonal)
                vdec = epool.tile([128, B_LOC], f32, name="vdec")
                nc.vector.scalar_tensor_tensor(
                    vdec[:], in0=i2[:], scalar=0.1, in1=ps1[:],
                    op0=A.mult, op1=A.add)

                z2 = epool.tile([128, B_LOC], u8, name="z2")
                m2 = epool.tile([128, B_LOC], u8, name="m2")
                nc.vector.tensor_scalar(z2[:], vdec[:], 1.0, None, op0=A.is_gt)
                nc.vector.tensor_scalar(m2[:], r2[:], 0.0, None, op0=A.is_gt)
                # base = max(rho - 1, 0)  ==  max(rho - (rho>0), 0)
                # (bf16: rho is bf16 on input, and base IS the rho_new
                # output except where z_new spikes)
                base = epool.tile([128, B_LOC], bf16, name="base")
                nc.vector.tensor_scalar(base[:], r2[:], 1.0, 0.0,
                                        op0=A.subtract, op1=A.max)

                # v_new: spike reset, then refractory hold
                nc.vector.copy_predicated(vdec[:], z2[:], zero_sb[:])
                nc.vector.copy_predicated(vdec[:], m2[:], v2)
                # z_new: suppress spikes while refractory (uint8, final)
                nc.vector.copy_predicated(z2[:], m2[:], zero_u8[:])
                # rho_new = base, 5 where z_new
                nc.vector.copy_predicated(base[:], z2[:], five_sb[:])

                vo2 = opool.tile([128, B_LOC], bf16, name="vo2")
                nc.gpsimd.tensor_copy(vo2[:], vdec[:])

                nc.scalar.dma_start(io_d[:, pw], io2[:])
                nc.sync.dma_start(zo_d[:, pw], z2[:])
                nc.sync.dma_start(vo_d[:, pw], vo2[:])
                nc.scalar.dma_start(ro_d[:, pw], base[:])

    nc.compile()
    return nc


def _sw_act(x, dt=np.float32):
    """[B_LOC, K] -> [128, KT*B_LOC] with layout [p, kt, b]."""
    a = np.ascontiguousarray(x.T).reshape(KT1, 128, B_LOC).transpose(1, 0, 2)
    return np.ascontiguousarray(a).astype(dt).reshape(128, KT1 * B_LOC)


def _unsw(y):
    """[128, HT*B_LOC] ([p, ht, b]) -> [B_LOC, H]."""
    a = y.reshape(128, HT, B_LOC).transpose(1, 0, 2).reshape(H, B_LOC)
    return a.T


def _sw_w(WT, kt, dt=np.float32):
    """WT=[K,H] -> [128, HT*kt*128] with layout [p, ht, kt, c]."""
    a = WT.reshape(kt, 128, HT, 128)              # [k, p, h, c]
    return np.ascontiguousarray(
        a.transpose(1, 2, 0, 3)).astype(dt).reshape(128, HT * kt * 128)


def kernel(inp, z, v, i, rho, input_weights, recurrent_weights, g_coupling):
    inp = np.ascontiguousarray(inp, dtype=np.float32)
    z = np.ascontiguousarray(z, dtype=np.float32)
    v = np.ascontiguousarray(v, dtype=np.float32)
    i = np.ascontiguousarray(i, dtype=np.float32)
    rho = np.ascontiguousarray(rho, dtype=np.float32)

    if "nc" not in _cache:
        _cache["nc"] = build()
    nc = _cache["nc"]
    wkey = (id(input_weights), id(recurrent_weights), id(g_coupling))
    if _cache.get("wkey") != wkey:
        G = np.asarray(g_coupling, np.float32).T.copy()
        G[np.arange(H), np.arange(H)] += 0.9          # leak folded in
        WiT = np.ascontiguousarray(np.asarray(input_weights, np.float32).T)
        WrT = np.ascontiguousarray(np.asarray(recurrent_weights, np.float32).T)
        # Wi k-tiles 0..NKF-1 as fp8 [p, ht, kf, c] -> [128, HT*NKF, 128]
        w2f = np.ascontiguousarray(
            WiT[:NKF * 128].reshape(NKF, 128, HT, 128).transpose(1, 2, 0, 3)
        ).astype(nfp8).reshape(128, HT * NKF, 128)
        # z ships centered (z-0.5, halves fp8 quantization error); the
        # bias 0.5*colsum(Wr_fp8) folds into the i_new-path i stream
        zbias = 0.5 * WrT.astype(nfp8).astype(np.float32).sum(axis=0)
        _cache["w"] = (_sw_w(G, KT1), _sw_w(WiT, KT1, nbf16),
                       _sw_w(WrT, KT1, nfp8).reshape(128, HT * KT1, 128),
                       w2f, zbias)
        _cache["wkey"] = wkey
    w1, w2x, w2z, w2f, zbias = _cache["w"]

    in_maps = []
    for c in range(NCORES):
        s = slice(c * B_LOC, (c + 1) * B_LOC)
        in_maps.append({
            "vt": _sw_act(v[s]),
            "xt": _sw_act(inp[s], nbf16),
            "zt": _sw_act(z[s] - 0.5, nfp8).reshape(128, KT1, B_LOC),
            "xf": np.ascontiguousarray(
                inp[s, :NKF * 128].T.reshape(NKF, 128, B_LOC)
                .transpose(1, 0, 2)).astype(nfp8),
            "it": _sw_act(i[s], nbf16),
            "i2t": _sw_act(i[s] + zbias[None, :] / 0.8, nbf16),
            "rt": _sw_act(rho[s], nbf16),
            "w1": w1, "w2x": w2x, "w2z": w2z, "w2f": w2f,
        })

    res = bass_utils.run_bass_kernel_spmd(
        nc, in_maps, core_ids=list(range(NCORES)),
        trace=bool(int(os.environ.get("LIF_TRACE", "0"))),
    )
    _cache["last_results"] = res

    outs = []
    for name in ["zo", "vo", "io", "ro"]:
        full = np.empty((B, H), np.float32)
        for c in range(NCORES):
            full[c * B_LOC:(c + 1) * B_LOC] = _unsw(
                res.results[c][name].astype(np.float32))
        outs.append(full)
    return np.stack(outs)
